# revision 26
# baseline (speedup 1.0000x reference)
"""MatchLSTM Trainium2 kernel: data-parallel over batch (8 cores, 1 batch elem each).

Per-core program (B=1), latency-optimized column-space recurrences:
- Embedding gather -> transposed input projections XP (row layout, bf16).
- GRU steps keep hidden state as columns (h-dim on partitions, 128+22 chunks).
  All recurrence matmuls have free size 1 (cost ~free on PE). Gates:
  one merged sigmoid over 4 psum columns, fused tanh(r*hn + xn) via
  activation(scale=r, bias=xn), blend h' = z*(h-n)+n via scalar_tensor_tensor.
  bf16 state written by DVE, fp32 canonical state by GpSimd, in parallel.
- Match recurrence: s = Wr^T h_m + Wp^T h_c as columns; G^T = tanh(whqT + s)
  via per-partition bias; attn = G^T^T w via 2 tiny matmuls; gate input
  projection folds the attention term through MT = Hq @ m_Wih[:,h:].T.
- q-GRU (64 steps) interleaved with the first 64 ctx-GRU steps; remaining
  ctx steps run standalone; the match loop is emitted last (the Tile
  scheduler overlaps it with the ctx tail where dependencies allow).
- PSUM accumulation groups are kept contiguous per region: real HW
  corrupts interleaved or reopened groups (verified by microprobe).
"""
import math
from contextlib import ExitStack

import numpy as np
import ml_dtypes

import concourse.bacc as bacc
import concourse.bass as bass
import concourse.mybir as mybir
import concourse.tile as tile
from concourse.bass_utils import run_bass_kernel_spmd

F32 = mybir.dt.float32
BF16 = mybir.dt.bfloat16
I32 = mybir.dt.int32
AF = mybir.ActivationFunctionType
OP = mybir.AluOpType
BF = ml_dtypes.bfloat16

H = 150
D = 300
J = 64
V = 100000

# gate-chunk column ranges within the 450-wide (r,z,n) projection
R0, R1 = slice(0, 128), slice(128, 150)
Z0, Z1 = slice(150, 278), slice(278, 300)
N0, N1 = slice(300, 428), slice(428, 450)


def _chunks(n, c=128):
    return [min(c, n - i) for i in range(0, n, c)]


def build(T=400):
    NT = math.ceil(T / 128)
    tsz = _chunks(T)

    nc = bacc.Bacc("TRN2", target_bir_lowering=False, debug=False, num_devices=8)

    # ---- DRAM inputs ----
    dram = {}

    def din(name, shape, dt):
        dram[name] = nc.dram_tensor(name, list(shape), dt, kind="ExternalInput")
        return dram[name]

    E_d = din("E", [V, D], F32)
    cidx_d = din("ctx_idx", [128, NT], I32)
    qidx_d = din("q_idx", [J, 1], I32)
    Ifp_d = din("Ifp", [128, 128], F32)
    Ibf_d = din("Ibf", [128, 128], BF16)
    onesr_d = din("ones_row", [1, 512], BF16)
    wnames = []
    for g in ("q", "c"):
        wnames += [(f"WihT_{g}_0", (128, 450)), (f"WihT_{g}_1", (128, 450)),
                   (f"WihT_{g}_2", (45, 450)),
                   (f"WhhT_{g}_0", (128, 450)), (f"WhhT_{g}_1", (23, 450))]
    wnames += [("WhhT_m_0", (128, 450)), ("WhhT_m_1", (23, 450)),
               ("W2T_0", (128, 450)), ("W2T_1", (23, 450)),
               ("WcT_0", (128, 450)), ("WcT_1", (23, 450)),
               ("Wr_0", (128, 150)), ("Wr_1", (23, 150)),
               ("Wp_0", (128, 150)), ("Wp_1", (23, 150)),
               ("Wq_0", (128, 150)), ("Wq_1", (23, 150)),
               ("w_col_0", (128, 1)), ("w_col_1", (22, 1))]
    for n, s in wnames:
        din(n, s, BF16)
    hr_d = nc.dram_tensor("hr", [T + 1, H], F32, kind="ExternalOutput")

    with tile.TileContext(nc) as tc, ExitStack() as st:
        sb = st.enter_context(tc.tile_pool(name="sb", bufs=1))

        def sbt(name, shape, dt):
            return sb.tile(list(shape), dt, tag=name, name=name)

        # ---- persistent SBUF tiles ----
        W = {n: sbt(n, s, BF16) for n, s in wnames}
        Ifp = sbt("Ifp", (128, 128), F32)
        Ibf = sbt("Ibf", (128, 128), BF16)
        cidx = sbt("cidx", (128, NT), I32)
        qidx = sbt("qidx", (J, 1), I32)
        ec = [sbt(f"ec{g}", (128, D), F32) for g in range(NT)]
        eq = sbt("eq", (J, D), F32)
        ecT = [sbt("ecT0", (128, T), BF16), sbt("ecT1", (128, T), BF16),
               sbt("ecT2", (45, T), BF16)]
        eqT = [sbt("eqT0", (128, J), BF16), sbt("eqT1", (128, J), BF16),
               sbt("eqT2", (45, J), BF16)]
        XPc = [sbt(f"XPc{g}", (tsz[g], 450), BF16) for g in range(NT)]
        XPq = sbt("XPq", (J, 450), BF16)
        XPnT = [sbt("XPnT0", (128, T), F32), sbt("XPnT1", (22, T), F32)]
        XPqnT = [sbt("XPqnT0", (128, J), F32), sbt("XPqnT1", (22, J), F32)]
        # hidden-state columns: bf16 (matmul operand) + fp32 (canonical)
        HqC = [sbt("HqC0", (128, J + 1), BF16), sbt("HqC1", (23, J + 1), BF16)]
        HcC = [sbt("HcC0", (128, T + 1), BF16), sbt("HcC1", (23, T + 1), BF16)]
        HmC = [sbt("HmC0", (128, T + 1), BF16), sbt("HmC1", (23, T + 1), BF16)]
        HqF = [sbt("HqF0", (128, J + 1), F32), sbt("HqF1", (22, J + 1), F32)]
        HcF = [sbt("HcF0", (128, T + 1), F32), sbt("HcF1", (22, T + 1), F32)]
        HmF = [sbt("HmF0", (128, T + 1), F32), sbt("HmF1", (22, T + 1), F32)]
        whqT = [sbt("whqT0", (128, J), BF16), sbt("whqT1", (22, J), BF16)]
        MT = sbt("MT", (J, 450), BF16)
        # per-chain scratch
        scr = {}
        for g in ("q", "c", "m"):
            scr[g] = dict(
                sig=sbt(f"sig_{g}", (128, 4), F32),
                n0=sbt(f"n0_{g}", (128, 1), F32),
                n1=sbt(f"n1_{g}", (22, 1), F32),
                d0=sbt(f"d0_{g}", (128, 1), F32),
                d1=sbt(f"d1_{g}", (22, 1), F32),
            )
        s_sb = sbt("s_sb", (128, 2), F32)
        hn_sb = sbt("hn_sb", (128, 2), F32)
        narg0 = sbt("narg0", (128, 1), F32)
        narg1 = sbt("narg1", (22, 1), F32)
        gt0 = sbt("gt0", (128, J), BF16)
        gt1 = sbt("gt1", (22, J), BF16)
        attb = sbt("attb", (J, 1), BF16)
        zrow = sbt("zrow", (1, H), F32)
        hrR = [sbt(f"hrR{g}", (tsz[g], H), F32) for g in range(NT)]

        # ---- load constants / weights ----
        for n, _ in wnames:
            nc.sync.dma_start(W[n][:], dram[n].ap())
        nc.sync.dma_start(Ifp[:], Ifp_d.ap())
        nc.sync.dma_start(Ibf[:], Ibf_d.ap())
        nc.sync.dma_start(cidx[:], cidx_d.ap())
        nc.sync.dma_start(qidx[:], qidx_d.ap())

        # ---- init state ----
        for hc, hf, ncol in ((HqC, HqF, J + 1), (HcC, HcF, T + 1),
                             (HmC, HmF, T + 1)):
            nc.vector.memset(hc[0][:, 0:1], 0.0)
            nc.vector.memset(hc[1][0:22, 0:1], 0.0)
            nc.sync.dma_start(hc[1][22:23, 0:ncol], onesr_d.ap()[0:1, 0:ncol])
            nc.vector.memset(hf[0][:, 0:1], 0.0)
            nc.vector.memset(hf[1][:, 0:1], 0.0)
        nc.sync.dma_start(ecT[2][44:45, 0:T], onesr_d.ap()[0:1, 0:T])
        nc.sync.dma_start(eqT[2][44:45, 0:J], onesr_d.ap()[0:1, 0:J])
        nc.vector.memset(zrow[:], 0.0)
        nc.sync.dma_start(hr_d.ap()[0:1, :], zrow[0:1, :])

        # ---- gathers ----
        for g in range(NT):
            nc.gpsimd.indirect_dma_start(
                out=ec[g][:], out_offset=None, in_=E_d.ap(),
                in_offset=bass.IndirectOffsetOnAxis(ap=cidx[:, g:g + 1], axis=0))
        nc.gpsimd.indirect_dma_start(
            out=eq[:], out_offset=None, in_=E_d.ap(),
            in_offset=bass.IndirectOffsetOnAxis(ap=qidx[:, 0:1], axis=0))

        dch = [(0, 128), (128, 128), (256, 44)]  # d-chunks of embedding dim

        # ---- preamble: transposes + input projections ----
        with tc.tile_pool(name="pre_ps", bufs=2, space="PSUM") as pps, \
             tc.tile_pool(name="xp_ps", bufs=2, space="PSUM") as xps:
            for g in range(NT):
                toff = 128 * g
                for k, (doff, dsz) in enumerate(dch):
                    tp = pps.tile([128, 128], F32, tag="tp", name="tp")
                    nc.tensor.transpose(tp[0:dsz, 0:tsz[g]],
                                        ec[g][0:tsz[g], doff:doff + dsz],
                                        Ifp[0:tsz[g], 0:tsz[g]])
                    nc.scalar.copy(ecT[k][0:dsz, toff:toff + tsz[g]],
                                   tp[0:dsz, 0:tsz[g]])
            for k, (doff, dsz) in enumerate(dch):
                tp = pps.tile([128, 128], F32, tag="tp", name="tp")
                nc.tensor.transpose(tp[0:dsz, 0:J], eq[0:J, doff:doff + dsz],
                                    Ifp[0:J, 0:J])
                nc.scalar.copy(eqT[k][0:dsz, 0:J], tp[0:dsz, 0:J])
            # XPq = [eq;1] @ [WihT_q; bih]
            xq = xps.tile([J, 450], F32, tag="xp", name="xp")
            for k in range(3):
                ksz = [128, 128, 45][k]
                nc.tensor.matmul(xq[0:J, :], eqT[k][0:ksz, 0:J],
                                 W[f"WihT_q_{k}"][0:ksz, :],
                                 start=(k == 0), stop=(k == 2))
            nc.vector.tensor_copy(XPq[:], xq[0:J, :])
            for g in range(NT):
                xc = xps.tile([128, 450], F32, tag="xp", name="xp")
                for k in range(3):
                    ksz = [128, 128, 45][k]
                    nc.tensor.matmul(xc[0:tsz[g], :],
                                     ecT[k][0:ksz, 128 * g:128 * g + tsz[g]],
                                     W[f"WihT_c_{k}"][0:ksz, :],
                                     start=(k == 0), stop=(k == 2))
                nc.vector.tensor_copy(XPc[g][:], xc[0:tsz[g], :])
            # XPnT: transpose of the n-gate input projection (bias for tanh)
            for g in range(NT):
                toff = 128 * g
                tpn = pps.tile([128, 128], BF16, tag="tpn", name="tpn")
                nc.tensor.transpose(tpn[0:128, 0:tsz[g]],
                                    XPc[g][0:tsz[g], N0],
                                    Ibf[0:tsz[g], 0:tsz[g]])
                nc.scalar.copy(XPnT[0][:, toff:toff + tsz[g]],
                               tpn[0:128, 0:tsz[g]])
                tpn2 = pps.tile([128, 128], BF16, tag="tpn", name="tpn")
                nc.tensor.transpose(tpn2[0:22, 0:tsz[g]],
                                    XPc[g][0:tsz[g], N1],
                                    Ibf[0:tsz[g], 0:tsz[g]])
                nc.scalar.copy(XPnT[1][:, toff:toff + tsz[g]],
                               tpn2[0:22, 0:tsz[g]])
            tpq = pps.tile([128, 128], BF16, tag="tpn", name="tpn")
            nc.tensor.transpose(tpq[0:128, 0:J], XPq[0:J, N0], Ibf[0:J, 0:J])
            nc.scalar.copy(XPqnT[0][:, 0:J], tpq[0:128, 0:J])
            tpq2 = pps.tile([128, 128], BF16, tag="tpn", name="tpn")
            nc.tensor.transpose(tpq2[0:22, 0:J], XPq[0:J, N1], Ibf[0:J, 0:J])
            nc.scalar.copy(XPqnT[1][:, 0:J], tpq2[0:22, 0:J])

        # ---- recurrence psum (persistent for q/c chains) ----
        psP = st.enter_context(tc.tile_pool(name="psP", bufs=1, space="PSUM"))
        # cols: 0 r0, 1 z0, 2 r1(rows<22), 3 z1(rows<22), 4 hn0,
        # 5 hn1(rows<22), 6..9 sigmoid(r0,z0,r1,z1) (PSUM-resident)
        psp = {"q": psP.tile([128, 10], F32, tag="ps_q", name="ps_q"),
               "c": psP.tile([128, 10], F32, tag="ps_c", name="ps_c")}
        nc.vector.memset(psp["q"][:], 0.0)
        nc.vector.memset(psp["c"][:], 0.0)

        def mm(out, lhsT, rhs, start=False, stop=False):
            nc.tensor.matmul(out, lhsT, rhs, start=start, stop=stop,
                             skip_group_check=True)

        def gru_pe(g, t, XPt, K, pos, HC, W0, W1):
            """Emit the 10 tiny matmuls of one GRU step (column space)."""
            ps = psp[g]
            ca, cb = HC[0][:, t:t + 1], HC[1][:, t:t + 1]
            e = Ibf[0:K, pos:pos + 1]
            mm(ps[0:128, 0:1], XPt[0:K, R0], e, start=True)
            mm(ps[0:128, 0:1], W0[:, R0], ca)
            mm(ps[0:128, 0:1], W1[:, R0], cb, stop=True)
            mm(ps[0:128, 1:2], XPt[0:K, Z0], e, start=True)
            mm(ps[0:128, 1:2], W0[:, Z0], ca)
            mm(ps[0:128, 1:2], W1[:, Z0], cb, stop=True)
            mm(ps[0:22, 2:3], XPt[0:K, R1], e, start=True)
            mm(ps[0:22, 2:3], W0[:, R1], ca)
            mm(ps[0:22, 2:3], W1[:, R1], cb, stop=True)
            mm(ps[0:22, 3:4], XPt[0:K, Z1], e, start=True)
            mm(ps[0:22, 3:4], W0[:, Z1], ca)
            mm(ps[0:22, 3:4], W1[:, Z1], cb, stop=True)
            mm(ps[0:128, 4:5], W0[:, N0], ca, start=True)
            mm(ps[0:128, 4:5], W1[:, N0], cb, stop=True)
            mm(ps[0:22, 5:6], W0[:, N1], ca, start=True)
            mm(ps[0:22, 5:6], W1[:, N1], cb, stop=True)

        def gru_act(g, t, XnT):
            ps, r = psp[g], scr[g]
            nc.scalar.activation(r["sig"][:], ps[0:128, 0:4], AF.Sigmoid)
            nc.scalar.activation(r["n0"][:], ps[0:128, 4:5], AF.Tanh,
                                 bias=XnT[0][:, t:t + 1],
                                 scale=r["sig"][:, 0:1])
            nc.scalar.activation(r["n1"][:], ps[0:22, 5:6], AF.Tanh,
                                 bias=XnT[1][0:22, t:t + 1],
                                 scale=r["sig"][0:22, 2:3])

        mtail_zsrc = [None]

        def _zs(g):
            if mtail_zsrc[0] is not None:
                return (mtail_zsrc[0][0:128, 11:12], mtail_zsrc[0][0:22, 13:14])
            r = scr[g]
            return (r["sig"][:, 1:2], r["sig"][0:22, 3:4])

        def gru_tail(g, t, HC, HF):
            r = scr[g]
            nc.vector.tensor_tensor(out=r["d0"][:], in0=HF[0][:, t:t + 1],
                                    in1=r["n0"][:], op=OP.subtract)
            nc.vector.tensor_tensor(out=r["d1"][:], in0=HF[1][0:22, t:t + 1],
                                    in1=r["n1"][:], op=OP.subtract)
            z0s, z1s = _zs(g)
            nc.vector.scalar_tensor_tensor(
                out=HC[0][:, t + 1:t + 2], in0=r["d0"][:],
                scalar=z0s, in1=r["n0"][:],
                op0=OP.mult, op1=OP.add)
            nc.vector.scalar_tensor_tensor(
                out=HC[1][0:22, t + 1:t + 2], in0=r["d1"][:],
                scalar=z1s, in1=r["n1"][:],
                op0=OP.mult, op1=OP.add)

        def gru_pool(g, t, HF):
            # fp32 canonical state; same blend on DVE (Pool lacks this op).
            # Emitted after the bf16 writes: only needed late next step.
            r = scr[g]
            z0s, z1s = _zs(g)
            nc.vector.scalar_tensor_tensor(
                out=HF[0][:, t + 1:t + 2], in0=r["d0"][:],
                scalar=z0s, in1=r["n0"][:],
                op0=OP.mult, op1=OP.add)
            nc.vector.scalar_tensor_tensor(
                out=HF[1][0:22, t + 1:t + 2], in0=r["d1"][:],
                scalar=z1s, in1=r["n1"][:],
                op0=OP.mult, op1=OP.add)

        def ctx_step(t):
            g, pos = divmod(t, 128)
            gru_pe("c", t, XPc[g], tsz[g], pos, HcC,
                   W["WhhT_c_0"], W["WhhT_c_1"])
            gru_act("c", t, XPnT)
            gru_tail("c", t, HcC, HcF)
            gru_pool("c", t, HcF)

        # ---- prefix: q-GRU interleaved with ctx-GRU steps 0..J-1 ----
        for j in range(J):
            gru_pe("q", j, XPq, J, j, HqC, W["WhhT_q_0"], W["WhhT_q_1"])
            gru_act("q", j, XPqnT)
            gru_tail("q", j, HqC, HqF)
            gru_pool("q", j, HqF)
            ctx_step(j)

        # ---- interlude: whqT, MT ----
        with tc.tile_pool(name="ps_i", bufs=1, space="PSUM") as psi:
            pw = psi.tile([128, J], F32, tag="pw", name="pw")
            pw1 = psi.tile([22, J], F32, tag="pw1", name="pw1")
            pmt = psi.tile([J, 450], F32, tag="pmt", name="pmt")
            mm(pw[0:128, :], W["Wq_0"][:, 0:128], HqC[0][:, 1:J + 1],
               start=True)
            mm(pw[0:128, :], W["Wq_1"][0:22, 0:128], HqC[1][0:22, 1:J + 1],
               stop=True)
            nc.vector.tensor_copy(whqT[0][:], pw[0:128, :])
            mm(pw1[0:22, :], W["Wq_0"][:, 128:150], HqC[0][:, 1:J + 1],
               start=True)
            mm(pw1[0:22, :], W["Wq_1"][0:22, 128:150], HqC[1][0:22, 1:J + 1],
               stop=True)
            nc.vector.tensor_copy(whqT[1][:], pw1[0:22, :])
            mm(pmt[0:J, :], HqC[0][:, 1:J + 1], W["W2T_0"][:, :], start=True)
            mm(pmt[0:J, :], HqC[1][0:22, 1:J + 1], W["W2T_1"][0:22, :],
               stop=True)
            nc.vector.tensor_copy(MT[:], pmt[0:J, :])

        # ---- match loop psum ----
        psM_pool = st.enter_context(
            tc.tile_pool(name="psM", bufs=1, space="PSUM"))
        # cols: 0 r0, 1 z0, 2 r1, 3 z1, 4 hn0, 5 hn1, 6 xn0, 7 xn1, 8 s0,
        # 9 s1, 10..13 sigmoid(r0,z0,r1,z1) (PSUM-resident: faster ack)
        psm = psM_pool.tile([128, 14], F32, tag="psm", name="psm")
        psa = psM_pool.tile([J, 1], F32, tag="psa", name="psa")
        nc.vector.memset(psm[:], 0.0)

        rm = scr["m"]

        def match_step(i, with_ctx):
            hm0, hm1 = HmC[0][:, i:i + 1], HmC[1][:, i:i + 1]
            hc0, hc1 = HcC[0][:, i + 1:i + 2], HcC[1][:, i + 1:i + 2]
            hm1s = HmC[1][0:22, i:i + 1]
            hc1s = HcC[1][0:22, i + 1:i + 2]
            # --- PE: ctx step mms first (independent), then s, partials ---
            if with_ctx:
                t = J + i
                g, pos = divmod(t, 128)
                gru_pe("c", t, XPc[g], tsz[g], pos, HcC,
                       W["WhhT_c_0"], W["WhhT_c_1"])
            # s columns (Wr^T hm + Wp^T hc); chunk1 first so GT1 can start
            # off a free scalar-sized copy while chunk0 copies under GT1
            mm(psm[0:22, 9:10], W["Wr_0"][:, 128:150], hm0, start=True)
            mm(psm[0:22, 9:10], W["Wr_1"][0:22, 128:150], hm1s)
            mm(psm[0:22, 9:10], W["Wp_0"][:, 128:150], hc0)
            mm(psm[0:22, 9:10], W["Wp_1"][0:22, 128:150], hc1s, stop=True)
            mm(psm[0:128, 8:9], W["Wr_0"][:, 0:128], hm0, start=True)
            mm(psm[0:128, 8:9], W["Wr_1"][0:22, 0:128], hm1s)
            mm(psm[0:128, 8:9], W["Wp_0"][:, 0:128], hc0)
            mm(psm[0:128, 8:9], W["Wp_1"][0:22, 0:128], hc1s, stop=True)
            # hn: Whh_m^T hm (+bhh) -- contiguous closed groups
            mm(psm[0:128, 4:5], W["WhhT_m_0"][:, N0], hm0, start=True)
            mm(psm[0:128, 4:5], W["WhhT_m_1"][:, N0], hm1, stop=True)
            mm(psm[0:22, 5:6], W["WhhT_m_0"][:, N1], hm0, start=True)
            mm(psm[0:22, 5:6], W["WhhT_m_1"][:, N1], hm1, stop=True)

            # --- s -> sbuf: chunk1 (scalar-sized, ~free) then chunk0 ---
            nc.vector.tensor_copy(s_sb[0:22, 1:2], psm[0:22, 9:10])
            nc.vector.tensor_copy(s_sb[0:128, 0:1], psm[0:128, 8:9])
            # --- ACT: G^T = tanh(whqT + s) ---
            nc.scalar.activation(gt1[:], whqT[1][:, :], AF.Tanh,
                                 bias=s_sb[0:22, 1:2])
            nc.scalar.activation(gt0[:], whqT[0][:, :], AF.Tanh,
                                 bias=s_sb[:, 0:1])
            nc.scalar.copy(hn_sb[:], psm[0:128, 4:6])
            # --- PE: attn = G w ---
            mm(psa[0:J, 0:1], gt1[0:22, :], W["w_col_1"][:, 0:1], start=True)
            mm(psa[0:J, 0:1], gt0[:, :], W["w_col_0"][:, 0:1], stop=True)
            nc.vector.tensor_copy(attb[:], psa[0:J, 0:1])
            # --- ctx ACT between GT and match sigmoid (fills the gap) ---
            if with_ctx:
                gru_act("c", t, XPnT)
            # --- PE: gate projections (each region one contiguous group;
            # HW corrupts interleaved/reopened psum accumulation groups) ---
            for col, cs in ((0, R0), (1, Z0), (2, R1), (3, Z1)):
                po = psm[0:128, col:col + 1] if col < 2 else \
                    psm[0:22, col:col + 1]
                mm(po, W["WcT_0"][:, cs], hc0, start=True)
                mm(po, W["WcT_1"][:, cs], hc1)
                mm(po, W["WhhT_m_0"][:, cs], hm0)
                mm(po, W["WhhT_m_1"][:, cs], hm1)
                mm(po, MT[:, cs], attb, stop=True)
            mm(psm[0:128, 6:7], W["WcT_0"][:, N0], hc0, start=True)
            mm(psm[0:128, 6:7], W["WcT_1"][:, N0], hc1)
            mm(psm[0:128, 6:7], MT[:, N0], attb, stop=True)
            mm(psm[0:22, 7:8], W["WcT_0"][:, N1], hc0, start=True)
            mm(psm[0:22, 7:8], W["WcT_1"][:, N1], hc1)
            mm(psm[0:22, 7:8], MT[:, N1], attb, stop=True)
            # --- gates: sigmoid stays in PSUM (143ns ack vs 185);
            # narg reads sig+xn from PSUM (dual-psum stt is HW-verified) ---
            nc.scalar.activation(psm[0:128, 10:14], psm[0:128, 0:4],
                                 AF.Sigmoid)
            if with_ctx:
                gru_tail("c", t, HcC, HcF)
                gru_pool("c", t, HcF)
            # chunk1 first end-to-end: next step's s1 matmuls gate GT1,
            # so the 22-row path must clear the DVE/ACT queues first
            nc.vector.scalar_tensor_tensor(
                out=narg1[:], in0=hn_sb[0:22, 1:2],
                scalar=psm[0:22, 12:13], in1=psm[0:22, 7:8],
                op0=OP.mult, op1=OP.add)
            nc.vector.scalar_tensor_tensor(
                out=narg0[:], in0=hn_sb[:, 0:1], scalar=psm[0:128, 10:11],
                in1=psm[0:128, 6:7], op0=OP.mult, op1=OP.add)
            nc.scalar.activation(rm["n1"][:], narg1[:], AF.Tanh)
            nc.scalar.activation(rm["n0"][:], narg0[:], AF.Tanh)
            r = rm
            nc.vector.tensor_tensor(out=r["d1"][:], in0=HmF[1][0:22, i:i + 1],
                                    in1=r["n1"][:], op=OP.subtract)
            nc.vector.scalar_tensor_tensor(
                out=HmC[1][0:22, i + 1:i + 2], in0=r["d1"][:],
                scalar=psm[0:22, 13:14], in1=r["n1"][:],
                op0=OP.mult, op1=OP.add)
            nc.vector.tensor_tensor(out=r["d0"][:], in0=HmF[0][:, i:i + 1],
                                    in1=r["n0"][:], op=OP.subtract)
            nc.vector.scalar_tensor_tensor(
                out=HmC[0][:, i + 1:i + 2], in0=r["d0"][:],
                scalar=psm[0:128, 11:12], in1=r["n0"][:],
                op0=OP.mult, op1=OP.add)
            nc.vector.scalar_tensor_tensor(
                out=HmF[1][0:22, i + 1:i + 2], in0=r["d1"][:],
                scalar=psm[0:22, 13:14], in1=r["n1"][:],
                op0=OP.mult, op1=OP.add)
            nc.vector.scalar_tensor_tensor(
                out=HmF[0][:, i + 1:i + 2], in0=r["d0"][:],
                scalar=psm[0:128, 11:12], in1=r["n0"][:],
                op0=OP.mult, op1=OP.add)

        # run the remaining ctx-GRU steps standalone (a lone GRU step is
        # ~5x cheaper than its scheduling wedge inside the match chain)
        for t in range(J, T):
            ctx_step(t)
        for i in range(T):
            match_step(i, with_ctx=False)

        # ---- postamble: transpose HmF columns 1..T into rows, DMA out ----
        with tc.tile_pool(name="ps_o", bufs=2, space="PSUM") as pso:
            for g in range(NT):
                cs = slice(1 + 128 * g, 1 + 128 * g + tsz[g])
                po = pso.tile([128, 150], F32, tag="po", name="po")
                nc.tensor.transpose(po[0:tsz[g], 0:128], HmF[0][:, cs],
                                    Ifp[0:128, 0:128])
                nc.tensor.transpose(po[0:tsz[g], 128:150], HmF[1][0:22, cs],
                                    Ifp[0:22, 0:22])
                nc.vector.tensor_copy(hrR[g][:], po[0:tsz[g], 0:150])
                nc.sync.dma_start(
                    hr_d.ap()[1 + 128 * g:1 + 128 * g + tsz[g], :],
                    hrR[g][0:tsz[g], :])

    nc.compile()
    return nc


def _bf(x):
    return np.ascontiguousarray(np.asarray(x, np.float32)).astype(BF)


def prep_shared(E, Wq, Wp, Wr, w, ctx_Wih, ctx_Whh, ctx_bih, ctx_bhh,
                q_Wih, q_Whh, q_bih, q_bhh, m_Wih, m_Whh, m_bih, m_bhh):
    f = {}
    f["Ifp"] = np.eye(128, dtype=np.float32)
    f["Ibf"] = _bf(np.eye(128))
    f["ones_row"] = _bf(np.ones((1, 512)))

    def wih_chunks(pfx, Wih, bih):
        WT = np.asarray(Wih, np.float32).T  # [300, 450]
        f[f"WihT_{pfx}_0"] = _bf(WT[0:128])
        f[f"WihT_{pfx}_1"] = _bf(WT[128:256])
        f[f"WihT_{pfx}_2"] = _bf(np.vstack([WT[256:300],
                                            np.asarray(bih, np.float32)[None, :]]))

    def whh_chunks(pfx, Whh, bhh):
        WT = np.asarray(Whh, np.float32).T  # [150, 450]
        f[f"WhhT_{pfx}_0"] = _bf(WT[0:128])
        f[f"WhhT_{pfx}_1"] = _bf(np.vstack([WT[128:150],
                                            np.asarray(bhh, np.float32)[None, :]]))

    def sq_chunks(pfx, M, brow=None):
        M = np.asarray(M, np.float32)
        if brow is None:
            brow = np.zeros((1, M.shape[1]), np.float32)
        f[f"{pfx}_0"] = _bf(M[0:128])
        f[f"{pfx}_1"] = _bf(np.vstack([M[128:150], brow]))

    wih_chunks("q", q_Wih, q_bih)
    wih_chunks("c", ctx_Wih, ctx_bih)
    whh_chunks("q", q_Whh, q_bhh)
    whh_chunks("c", ctx_Whh, ctx_bhh)
    whh_chunks("m", m_Whh, m_bhh)
    m_Wih = np.asarray(m_Wih, np.float32)
    # m_bih rides on WcT's bias row (paired with the ones row of HcC chunk 1)
    sq_chunks("WcT", m_Wih[:, :H].T, np.asarray(m_bih, np.float32)[None, :])
    sq_chunks("W2T", m_Wih[:, H:].T)
    sq_chunks("Wr", np.asarray(Wr, np.float32))
    sq_chunks("Wp", np.asarray(Wp, np.float32))
    sq_chunks("Wq", np.asarray(Wq, np.float32))
    wf = np.asarray(w, np.float32)
    f["w_col_0"] = _bf(wf[0:128][:, None])
    f["w_col_1"] = _bf(wf[128:150][:, None])
    return f


_NC_CACHE = {}


def kernel(context, query, E, Wq, Wp, Wr, w, ctx_Wih, ctx_Whh, ctx_bih,
           ctx_bhh, q_Wih, q_Whh, q_bih, q_bhh, m_Wih, m_Whh, m_bih, m_bhh,
           _T=None):
    context = np.asarray(context)
    query = np.asarray(query)
    B, T = context.shape
    if _T is not None:
        T = _T
        context = context[:, :T]
    NT = math.ceil(T / 128)
    if T not in _NC_CACHE:
        _NC_CACHE[T] = build(T)
    nc = _NC_CACHE[T]

    shared = prep_shared(E, Wq, Wp, Wr, w, ctx_Wih, ctx_Whh, ctx_bih, ctx_bhh,
                         q_Wih, q_Whh, q_bih, q_bhh, m_Wih, m_Whh, m_bih, m_bhh)
    E_np = np.ascontiguousarray(np.asarray(E, np.float32))
    in_maps = []
    for b in range(B):
        m = dict(shared)
        m["E"] = E_np
        ci = np.zeros((128, NT), np.int32)
        flat = np.asarray(context[b], np.int64).astype(np.int32)
        for g in range(NT):
            n = min(128, T - 128 * g)
            ci[0:n, g] = flat[128 * g:128 * g + n]
        m["ctx_idx"] = ci
        m["q_idx"] = np.asarray(query[b], np.int64).astype(np.int32)[:, None]
        in_maps.append(m)

    res = run_bass_kernel_spmd(nc, in_maps, core_ids=list(range(B)))
    out = np.stack([r["hr"] for r in res.results], axis=0)
    return out.astype(np.float32)


# revision 27
# speedup vs baseline: 1.0049x; 1.0049x over previous
"""MatchLSTM Trainium2 kernel: data-parallel over batch (8 cores, 1 batch elem each).

Per-core program (B=1), latency-optimized column-space recurrences:
- Embedding gather -> transposed input projections XP (row layout, bf16).
- GRU steps keep hidden state as columns (h-dim on partitions, 128+22 chunks).
  All recurrence matmuls have free size 1 (cost ~free on PE). Gates:
  one merged sigmoid over 4 psum columns, fused tanh(r*hn + xn) via
  activation(scale=r, bias=xn), blend h' = z*(h-n)+n via scalar_tensor_tensor.
  bf16 state written by DVE, fp32 canonical state by GpSimd, in parallel.
- Match recurrence: s = Wr^T h_m + Wp^T h_c as columns; G^T = tanh(whqT + s)
  via per-partition bias; attn = G^T^T w via 2 tiny matmuls; gate input
  projection folds the attention term through MT = Hq @ m_Wih[:,h:].T.
- q-GRU (64 steps) interleaved with the first 64 ctx-GRU steps; remaining
  ctx steps run standalone; the match loop is emitted last (the Tile
  scheduler overlaps it with the ctx tail where dependencies allow).
- PSUM accumulation groups are kept contiguous per region: real HW
  corrupts interleaved or reopened groups (verified by microprobe).
"""
import math
from contextlib import ExitStack

import numpy as np
import ml_dtypes

import concourse.bacc as bacc
import concourse.bass as bass
import concourse.mybir as mybir
import concourse.tile as tile
from concourse.bass_utils import run_bass_kernel_spmd

F32 = mybir.dt.float32
BF16 = mybir.dt.bfloat16
I32 = mybir.dt.int32
AF = mybir.ActivationFunctionType
OP = mybir.AluOpType
BF = ml_dtypes.bfloat16

H = 150
D = 300
J = 64
V = 100000

# gate-chunk column ranges within the 450-wide (r,z,n) projection
R0, R1 = slice(0, 128), slice(128, 150)
Z0, Z1 = slice(150, 278), slice(278, 300)
N0, N1 = slice(300, 428), slice(428, 450)


def _chunks(n, c=128):
    return [min(c, n - i) for i in range(0, n, c)]


def build(T=400):
    NT = math.ceil(T / 128)
    tsz = _chunks(T)

    nc = bacc.Bacc("TRN2", target_bir_lowering=False, debug=False, num_devices=8)

    # ---- DRAM inputs ----
    dram = {}

    def din(name, shape, dt):
        dram[name] = nc.dram_tensor(name, list(shape), dt, kind="ExternalInput")
        return dram[name]

    E_d = din("E", [V, D], F32)
    cidx_d = din("ctx_idx", [128, NT], I32)
    qidx_d = din("q_idx", [J, 1], I32)
    Ifp_d = din("Ifp", [128, 128], F32)
    Ibf_d = din("Ibf", [128, 128], BF16)
    onesr_d = din("ones_row", [1, 512], BF16)
    wnames = []
    for g in ("q", "c"):
        wnames += [(f"WihT_{g}_0", (128, 450)), (f"WihT_{g}_1", (128, 450)),
                   (f"WihT_{g}_2", (45, 450)),
                   (f"WhhT_{g}_0", (128, 450)), (f"WhhT_{g}_1", (23, 450))]
    wnames += [("WhhT_m_0", (128, 450)), ("WhhT_m_1", (23, 450)),
               ("W2T_0", (128, 450)), ("W2T_1", (23, 450)),
               ("WcT_0", (128, 450)), ("WcT_1", (23, 450)),
               ("Wr_0", (128, 150)), ("Wr_1", (23, 150)),
               ("Wp_0", (128, 150)), ("Wp_1", (23, 150)),
               ("Wq_0", (128, 150)), ("Wq_1", (23, 150)),
               ("w_col_0", (128, 1)), ("w_col_1", (22, 1))]
    for n, s in wnames:
        din(n, s, BF16)
    hr_d = nc.dram_tensor("hr", [T + 1, H], F32, kind="ExternalOutput")

    with tile.TileContext(nc) as tc, ExitStack() as st:
        sb = st.enter_context(tc.tile_pool(name="sb", bufs=1))

        def sbt(name, shape, dt):
            return sb.tile(list(shape), dt, tag=name, name=name)

        # ---- persistent SBUF tiles ----
        W = {n: sbt(n, s, BF16) for n, s in wnames}
        Ifp = sbt("Ifp", (128, 128), F32)
        Ibf = sbt("Ibf", (128, 128), BF16)
        cidx = sbt("cidx", (128, NT), I32)
        qidx = sbt("qidx", (J, 1), I32)
        ec = [sbt(f"ec{g}", (128, D), F32) for g in range(NT)]
        eq = sbt("eq", (J, D), F32)
        ecT = [sbt("ecT0", (128, T), BF16), sbt("ecT1", (128, T), BF16),
               sbt("ecT2", (45, T), BF16)]
        eqT = [sbt("eqT0", (128, J), BF16), sbt("eqT1", (128, J), BF16),
               sbt("eqT2", (45, J), BF16)]
        XPc = [sbt(f"XPc{g}", (tsz[g], 450), BF16) for g in range(NT)]
        XPq = sbt("XPq", (J, 450), BF16)
        XPnT = [sbt("XPnT0", (128, T), F32), sbt("XPnT1", (22, T), F32)]
        XPqnT = [sbt("XPqnT0", (128, J), F32), sbt("XPqnT1", (22, J), F32)]
        # hidden-state columns: bf16 (matmul operand) + fp32 (canonical)
        HqC = [sbt("HqC0", (128, J + 1), BF16), sbt("HqC1", (23, J + 1), BF16)]
        HcC = [sbt("HcC0", (128, T + 1), BF16), sbt("HcC1", (23, T + 1), BF16)]
        HmC = [sbt("HmC0", (128, T + 1), BF16), sbt("HmC1", (23, T + 1), BF16)]
        HqF = [sbt("HqF0", (128, J + 1), F32), sbt("HqF1", (22, J + 1), F32)]
        HcF = [sbt("HcF0", (128, T + 1), F32), sbt("HcF1", (22, T + 1), F32)]
        HmF = [sbt("HmF0", (128, T + 1), F32), sbt("HmF1", (22, T + 1), F32)]
        whqT = [sbt("whqT0", (128, J), BF16), sbt("whqT1", (22, J), BF16)]
        MT = sbt("MT", (J, 450), BF16)
        # per-chain scratch
        scr = {}
        for g in ("q", "c", "m"):
            scr[g] = dict(
                sig=sbt(f"sig_{g}", (128, 4), F32),
                n0=sbt(f"n0_{g}", (128, 1), F32),
                n1=sbt(f"n1_{g}", (22, 1), F32),
                d0=sbt(f"d0_{g}", (128, 1), F32),
                d1=sbt(f"d1_{g}", (22, 1), F32),
            )
        s_sb = sbt("s_sb", (128, 2), F32)
        hn_sb = sbt("hn_sb", (128, 2), F32)
        narg0 = sbt("narg0", (128, 1), F32)
        narg1 = sbt("narg1", (22, 1), F32)
        gt0 = sbt("gt0", (128, J), BF16)
        gt1 = sbt("gt1", (22, J), BF16)
        attb = sbt("attb", (J, 1), BF16)
        zrow = sbt("zrow", (1, H), F32)
        hrR = [sbt(f"hrR{g}", (tsz[g], H), F32) for g in range(NT)]

        # ---- load constants / weights (indices first: gathers wait on
        # them, and the SP issue queue serializes DMA starts) ----
        nc.sync.dma_start(cidx[:], cidx_d.ap())
        nc.sync.dma_start(qidx[:], qidx_d.ap())
        nc.sync.dma_start(Ifp[:], Ifp_d.ap())
        nc.sync.dma_start(Ibf[:], Ibf_d.ap())
        for n, _ in wnames:
            nc.sync.dma_start(W[n][:], dram[n].ap())

        # ---- init state ----
        for hc, hf, ncol in ((HqC, HqF, J + 1), (HcC, HcF, T + 1),
                             (HmC, HmF, T + 1)):
            nc.vector.memset(hc[0][:, 0:1], 0.0)
            nc.vector.memset(hc[1][0:22, 0:1], 0.0)
            nc.sync.dma_start(hc[1][22:23, 0:ncol], onesr_d.ap()[0:1, 0:ncol])
            nc.vector.memset(hf[0][:, 0:1], 0.0)
            nc.vector.memset(hf[1][:, 0:1], 0.0)
        nc.sync.dma_start(ecT[2][44:45, 0:T], onesr_d.ap()[0:1, 0:T])
        nc.sync.dma_start(eqT[2][44:45, 0:J], onesr_d.ap()[0:1, 0:J])
        nc.vector.memset(zrow[:], 0.0)
        nc.sync.dma_start(hr_d.ap()[0:1, :], zrow[0:1, :])

        # ---- gathers ----
        for g in range(NT):
            nc.gpsimd.indirect_dma_start(
                out=ec[g][:], out_offset=None, in_=E_d.ap(),
                in_offset=bass.IndirectOffsetOnAxis(ap=cidx[:, g:g + 1], axis=0))
        nc.gpsimd.indirect_dma_start(
            out=eq[:], out_offset=None, in_=E_d.ap(),
            in_offset=bass.IndirectOffsetOnAxis(ap=qidx[:, 0:1], axis=0))

        dch = [(0, 128), (128, 128), (256, 44)]  # d-chunks of embedding dim

        # ---- preamble: transposes + input projections ----
        with tc.tile_pool(name="pre_ps", bufs=2, space="PSUM") as pps, \
             tc.tile_pool(name="xp_ps", bufs=2, space="PSUM") as xps:
            for g in range(NT):
                toff = 128 * g
                for k, (doff, dsz) in enumerate(dch):
                    tp = pps.tile([128, 128], F32, tag="tp", name="tp")
                    nc.tensor.transpose(tp[0:dsz, 0:tsz[g]],
                                        ec[g][0:tsz[g], doff:doff + dsz],
                                        Ifp[0:tsz[g], 0:tsz[g]])
                    nc.scalar.copy(ecT[k][0:dsz, toff:toff + tsz[g]],
                                   tp[0:dsz, 0:tsz[g]])
            for k, (doff, dsz) in enumerate(dch):
                tp = pps.tile([128, 128], F32, tag="tp", name="tp")
                nc.tensor.transpose(tp[0:dsz, 0:J], eq[0:J, doff:doff + dsz],
                                    Ifp[0:J, 0:J])
                nc.scalar.copy(eqT[k][0:dsz, 0:J], tp[0:dsz, 0:J])
            # XPq = [eq;1] @ [WihT_q; bih]
            xq = xps.tile([J, 450], F32, tag="xp", name="xp")
            for k in range(3):
                ksz = [128, 128, 45][k]
                nc.tensor.matmul(xq[0:J, :], eqT[k][0:ksz, 0:J],
                                 W[f"WihT_q_{k}"][0:ksz, :],
                                 start=(k == 0), stop=(k == 2))
            nc.vector.tensor_copy(XPq[:], xq[0:J, :])
            for g in range(NT):
                xc = xps.tile([128, 450], F32, tag="xp", name="xp")
                for k in range(3):
                    ksz = [128, 128, 45][k]
                    nc.tensor.matmul(xc[0:tsz[g], :],
                                     ecT[k][0:ksz, 128 * g:128 * g + tsz[g]],
                                     W[f"WihT_c_{k}"][0:ksz, :],
                                     start=(k == 0), stop=(k == 2))
                nc.vector.tensor_copy(XPc[g][:], xc[0:tsz[g], :])
            # XPnT: transpose of the n-gate input projection (bias for tanh)
            for g in range(NT):
                toff = 128 * g
                tpn = pps.tile([128, 128], BF16, tag="tpn", name="tpn")
                nc.tensor.transpose(tpn[0:128, 0:tsz[g]],
                                    XPc[g][0:tsz[g], N0],
                                    Ibf[0:tsz[g], 0:tsz[g]])
                nc.scalar.copy(XPnT[0][:, toff:toff + tsz[g]],
                               tpn[0:128, 0:tsz[g]])
                tpn2 = pps.tile([128, 128], BF16, tag="tpn", name="tpn")
                nc.tensor.transpose(tpn2[0:22, 0:tsz[g]],
                                    XPc[g][0:tsz[g], N1],
                                    Ibf[0:tsz[g], 0:tsz[g]])
                nc.scalar.copy(XPnT[1][:, toff:toff + tsz[g]],
                               tpn2[0:22, 0:tsz[g]])
            tpq = pps.tile([128, 128], BF16, tag="tpn", name="tpn")
            nc.tensor.transpose(tpq[0:128, 0:J], XPq[0:J, N0], Ibf[0:J, 0:J])
            nc.scalar.copy(XPqnT[0][:, 0:J], tpq[0:128, 0:J])
            tpq2 = pps.tile([128, 128], BF16, tag="tpn", name="tpn")
            nc.tensor.transpose(tpq2[0:22, 0:J], XPq[0:J, N1], Ibf[0:J, 0:J])
            nc.scalar.copy(XPqnT[1][:, 0:J], tpq2[0:22, 0:J])

        # ---- recurrence psum (persistent for q/c chains) ----
        psP = st.enter_context(tc.tile_pool(name="psP", bufs=1, space="PSUM"))
        # cols: 0 r0, 1 z0, 2 r1(rows<22), 3 z1(rows<22), 4 hn0,
        # 5 hn1(rows<22), 6..9 sigmoid(r0,z0,r1,z1) (PSUM-resident)
        psp = {"q": psP.tile([128, 10], F32, tag="ps_q", name="ps_q"),
               "c": psP.tile([128, 10], F32, tag="ps_c", name="ps_c")}
        nc.vector.memset(psp["q"][:], 0.0)
        nc.vector.memset(psp["c"][:], 0.0)

        def mm(out, lhsT, rhs, start=False, stop=False):
            nc.tensor.matmul(out, lhsT, rhs, start=start, stop=stop,
                             skip_group_check=True)

        def gru_pe(g, t, XPt, K, pos, HC, W0, W1):
            """Emit the 10 tiny matmuls of one GRU step (column space)."""
            ps = psp[g]
            ca, cb = HC[0][:, t:t + 1], HC[1][:, t:t + 1]
            e = Ibf[0:K, pos:pos + 1]
            mm(ps[0:128, 0:1], XPt[0:K, R0], e, start=True)
            mm(ps[0:128, 0:1], W0[:, R0], ca)
            mm(ps[0:128, 0:1], W1[:, R0], cb, stop=True)
            mm(ps[0:128, 1:2], XPt[0:K, Z0], e, start=True)
            mm(ps[0:128, 1:2], W0[:, Z0], ca)
            mm(ps[0:128, 1:2], W1[:, Z0], cb, stop=True)
            mm(ps[0:22, 2:3], XPt[0:K, R1], e, start=True)
            mm(ps[0:22, 2:3], W0[:, R1], ca)
            mm(ps[0:22, 2:3], W1[:, R1], cb, stop=True)
            mm(ps[0:22, 3:4], XPt[0:K, Z1], e, start=True)
            mm(ps[0:22, 3:4], W0[:, Z1], ca)
            mm(ps[0:22, 3:4], W1[:, Z1], cb, stop=True)
            mm(ps[0:128, 4:5], W0[:, N0], ca, start=True)
            mm(ps[0:128, 4:5], W1[:, N0], cb, stop=True)
            mm(ps[0:22, 5:6], W0[:, N1], ca, start=True)
            mm(ps[0:22, 5:6], W1[:, N1], cb, stop=True)

        def gru_act(g, t, XnT):
            ps, r = psp[g], scr[g]
            nc.scalar.activation(r["sig"][:], ps[0:128, 0:4], AF.Sigmoid)
            nc.scalar.activation(r["n0"][:], ps[0:128, 4:5], AF.Tanh,
                                 bias=XnT[0][:, t:t + 1],
                                 scale=r["sig"][:, 0:1])
            nc.scalar.activation(r["n1"][:], ps[0:22, 5:6], AF.Tanh,
                                 bias=XnT[1][0:22, t:t + 1],
                                 scale=r["sig"][0:22, 2:3])

        mtail_zsrc = [None]

        def _zs(g):
            if mtail_zsrc[0] is not None:
                return (mtail_zsrc[0][0:128, 11:12], mtail_zsrc[0][0:22, 13:14])
            r = scr[g]
            return (r["sig"][:, 1:2], r["sig"][0:22, 3:4])

        def gru_tail(g, t, HC, HF):
            r = scr[g]
            nc.vector.tensor_tensor(out=r["d0"][:], in0=HF[0][:, t:t + 1],
                                    in1=r["n0"][:], op=OP.subtract)
            nc.vector.tensor_tensor(out=r["d1"][:], in0=HF[1][0:22, t:t + 1],
                                    in1=r["n1"][:], op=OP.subtract)
            z0s, z1s = _zs(g)
            nc.vector.scalar_tensor_tensor(
                out=HC[0][:, t + 1:t + 2], in0=r["d0"][:],
                scalar=z0s, in1=r["n0"][:],
                op0=OP.mult, op1=OP.add)
            nc.vector.scalar_tensor_tensor(
                out=HC[1][0:22, t + 1:t + 2], in0=r["d1"][:],
                scalar=z1s, in1=r["n1"][:],
                op0=OP.mult, op1=OP.add)

        def gru_pool(g, t, HF):
            # fp32 canonical state; same blend on DVE (Pool lacks this op).
            # Emitted after the bf16 writes: only needed late next step.
            r = scr[g]
            z0s, z1s = _zs(g)
            nc.vector.scalar_tensor_tensor(
                out=HF[0][:, t + 1:t + 2], in0=r["d0"][:],
                scalar=z0s, in1=r["n0"][:],
                op0=OP.mult, op1=OP.add)
            nc.vector.scalar_tensor_tensor(
                out=HF[1][0:22, t + 1:t + 2], in0=r["d1"][:],
                scalar=z1s, in1=r["n1"][:],
                op0=OP.mult, op1=OP.add)

        def ctx_step(t):
            g, pos = divmod(t, 128)
            gru_pe("c", t, XPc[g], tsz[g], pos, HcC,
                   W["WhhT_c_0"], W["WhhT_c_1"])
            gru_act("c", t, XPnT)
            gru_tail("c", t, HcC, HcF)
            gru_pool("c", t, HcF)

        # ---- prefix: q-GRU interleaved with ctx-GRU steps 0..J-1 ----
        for j in range(J):
            gru_pe("q", j, XPq, J, j, HqC, W["WhhT_q_0"], W["WhhT_q_1"])
            gru_act("q", j, XPqnT)
            gru_tail("q", j, HqC, HqF)
            gru_pool("q", j, HqF)
            ctx_step(j)

        # ---- interlude: whqT, MT ----
        with tc.tile_pool(name="ps_i", bufs=1, space="PSUM") as psi:
            pw = psi.tile([128, J], F32, tag="pw", name="pw")
            pw1 = psi.tile([22, J], F32, tag="pw1", name="pw1")
            pmt = psi.tile([J, 450], F32, tag="pmt", name="pmt")
            mm(pw[0:128, :], W["Wq_0"][:, 0:128], HqC[0][:, 1:J + 1],
               start=True)
            mm(pw[0:128, :], W["Wq_1"][0:22, 0:128], HqC[1][0:22, 1:J + 1],
               stop=True)
            nc.vector.tensor_copy(whqT[0][:], pw[0:128, :])
            mm(pw1[0:22, :], W["Wq_0"][:, 128:150], HqC[0][:, 1:J + 1],
               start=True)
            mm(pw1[0:22, :], W["Wq_1"][0:22, 128:150], HqC[1][0:22, 1:J + 1],
               stop=True)
            nc.vector.tensor_copy(whqT[1][:], pw1[0:22, :])
            mm(pmt[0:J, :], HqC[0][:, 1:J + 1], W["W2T_0"][:, :], start=True)
            mm(pmt[0:J, :], HqC[1][0:22, 1:J + 1], W["W2T_1"][0:22, :],
               stop=True)
            nc.vector.tensor_copy(MT[:], pmt[0:J, :])

        # ---- match loop psum ----
        psM_pool = st.enter_context(
            tc.tile_pool(name="psM", bufs=1, space="PSUM"))
        # cols: 0 r0, 1 z0, 2 r1, 3 z1, 4 hn0, 5 hn1, 6 xn0, 7 xn1, 8 s0,
        # 9 s1, 10..13 sigmoid(r0,z0,r1,z1) (PSUM-resident: faster ack)
        psm = psM_pool.tile([128, 14], F32, tag="psm", name="psm")
        psa = psM_pool.tile([J, 1], F32, tag="psa", name="psa")
        nc.vector.memset(psm[:], 0.0)

        rm = scr["m"]

        def match_step(i, with_ctx):
            hm0, hm1 = HmC[0][:, i:i + 1], HmC[1][:, i:i + 1]
            hc0, hc1 = HcC[0][:, i + 1:i + 2], HcC[1][:, i + 1:i + 2]
            hm1s = HmC[1][0:22, i:i + 1]
            hc1s = HcC[1][0:22, i + 1:i + 2]
            # --- PE: ctx step mms first (independent), then s, partials ---
            if with_ctx:
                t = J + i
                g, pos = divmod(t, 128)
                gru_pe("c", t, XPc[g], tsz[g], pos, HcC,
                       W["WhhT_c_0"], W["WhhT_c_1"])
            # s columns (Wr^T hm + Wp^T hc); chunk1 first so GT1 can start
            # off a free scalar-sized copy while chunk0 copies under GT1
            mm(psm[0:22, 9:10], W["Wr_0"][:, 128:150], hm0, start=True)
            mm(psm[0:22, 9:10], W["Wr_1"][0:22, 128:150], hm1s)
            mm(psm[0:22, 9:10], W["Wp_0"][:, 128:150], hc0)
            mm(psm[0:22, 9:10], W["Wp_1"][0:22, 128:150], hc1s, stop=True)
            mm(psm[0:128, 8:9], W["Wr_0"][:, 0:128], hm0, start=True)
            mm(psm[0:128, 8:9], W["Wr_1"][0:22, 0:128], hm1s)
            mm(psm[0:128, 8:9], W["Wp_0"][:, 0:128], hc0)
            mm(psm[0:128, 8:9], W["Wp_1"][0:22, 0:128], hc1s, stop=True)
            # hn: Whh_m^T hm (+bhh) -- contiguous closed groups
            mm(psm[0:128, 4:5], W["WhhT_m_0"][:, N0], hm0, start=True)
            mm(psm[0:128, 4:5], W["WhhT_m_1"][:, N0], hm1, stop=True)
            mm(psm[0:22, 5:6], W["WhhT_m_0"][:, N1], hm0, start=True)
            mm(psm[0:22, 5:6], W["WhhT_m_1"][:, N1], hm1, stop=True)

            # --- s -> sbuf: chunk1 (scalar-sized, ~free) then chunk0 ---
            nc.vector.tensor_copy(s_sb[0:22, 1:2], psm[0:22, 9:10])
            nc.vector.tensor_copy(s_sb[0:128, 0:1], psm[0:128, 8:9])
            # --- ACT: G^T = tanh(whqT + s) ---
            nc.scalar.activation(gt1[:], whqT[1][:, :], AF.Tanh,
                                 bias=s_sb[0:22, 1:2])
            nc.scalar.activation(gt0[:], whqT[0][:, :], AF.Tanh,
                                 bias=s_sb[:, 0:1])
            nc.scalar.copy(hn_sb[:], psm[0:128, 4:6])
            # --- PE: attn = G w ---
            mm(psa[0:J, 0:1], gt1[0:22, :], W["w_col_1"][:, 0:1], start=True)
            mm(psa[0:J, 0:1], gt0[:, :], W["w_col_0"][:, 0:1], stop=True)
            nc.vector.tensor_copy(attb[:], psa[0:J, 0:1])
            # --- ctx ACT between GT and match sigmoid (fills the gap) ---
            if with_ctx:
                gru_act("c", t, XPnT)
            # --- PE: gate projections (each region one contiguous group;
            # HW corrupts interleaved/reopened psum accumulation groups) ---
            for col, cs in ((0, R0), (1, Z0), (2, R1), (3, Z1)):
                po = psm[0:128, col:col + 1] if col < 2 else \
                    psm[0:22, col:col + 1]
                mm(po, W["WcT_0"][:, cs], hc0, start=True)
                mm(po, W["WcT_1"][:, cs], hc1)
                mm(po, W["WhhT_m_0"][:, cs], hm0)
                mm(po, W["WhhT_m_1"][:, cs], hm1)
                mm(po, MT[:, cs], attb, stop=True)
            mm(psm[0:128, 6:7], W["WcT_0"][:, N0], hc0, start=True)
            mm(psm[0:128, 6:7], W["WcT_1"][:, N0], hc1)
            mm(psm[0:128, 6:7], MT[:, N0], attb, stop=True)
            mm(psm[0:22, 7:8], W["WcT_0"][:, N1], hc0, start=True)
            mm(psm[0:22, 7:8], W["WcT_1"][:, N1], hc1)
            mm(psm[0:22, 7:8], MT[:, N1], attb, stop=True)
            # --- gates: sigmoid stays in PSUM (143ns ack vs 185);
            # narg reads sig+xn from PSUM (dual-psum stt is HW-verified) ---
            nc.scalar.activation(psm[0:128, 10:14], psm[0:128, 0:4],
                                 AF.Sigmoid)
            if with_ctx:
                gru_tail("c", t, HcC, HcF)
                gru_pool("c", t, HcF)
            # chunk1 first end-to-end: next step's s1 matmuls gate GT1,
            # so the 22-row path must clear the DVE/ACT queues first
            nc.vector.scalar_tensor_tensor(
                out=narg1[:], in0=hn_sb[0:22, 1:2],
                scalar=psm[0:22, 12:13], in1=psm[0:22, 7:8],
                op0=OP.mult, op1=OP.add)
            nc.vector.scalar_tensor_tensor(
                out=narg0[:], in0=hn_sb[:, 0:1], scalar=psm[0:128, 10:11],
                in1=psm[0:128, 6:7], op0=OP.mult, op1=OP.add)
            nc.scalar.activation(rm["n1"][:], narg1[:], AF.Tanh)
            nc.scalar.activation(rm["n0"][:], narg0[:], AF.Tanh)
            r = rm
            nc.vector.tensor_tensor(out=r["d1"][:], in0=HmF[1][0:22, i:i + 1],
                                    in1=r["n1"][:], op=OP.subtract)
            nc.vector.scalar_tensor_tensor(
                out=HmC[1][0:22, i + 1:i + 2], in0=r["d1"][:],
                scalar=psm[0:22, 13:14], in1=r["n1"][:],
                op0=OP.mult, op1=OP.add)
            nc.vector.tensor_tensor(out=r["d0"][:], in0=HmF[0][:, i:i + 1],
                                    in1=r["n0"][:], op=OP.subtract)
            nc.vector.scalar_tensor_tensor(
                out=HmC[0][:, i + 1:i + 2], in0=r["d0"][:],
                scalar=psm[0:128, 11:12], in1=r["n0"][:],
                op0=OP.mult, op1=OP.add)
            nc.vector.scalar_tensor_tensor(
                out=HmF[1][0:22, i + 1:i + 2], in0=r["d1"][:],
                scalar=psm[0:22, 13:14], in1=r["n1"][:],
                op0=OP.mult, op1=OP.add)
            nc.vector.scalar_tensor_tensor(
                out=HmF[0][:, i + 1:i + 2], in0=r["d0"][:],
                scalar=psm[0:128, 11:12], in1=r["n0"][:],
                op0=OP.mult, op1=OP.add)

        # run the remaining ctx-GRU steps standalone (a lone GRU step is
        # ~5x cheaper than its scheduling wedge inside the match chain)
        for t in range(J, T):
            ctx_step(t)
        for i in range(T):
            match_step(i, with_ctx=False)

        # ---- postamble: transpose HmF columns 1..T into rows, DMA out ----
        with tc.tile_pool(name="ps_o", bufs=2, space="PSUM") as pso:
            for g in range(NT):
                cs = slice(1 + 128 * g, 1 + 128 * g + tsz[g])
                po = pso.tile([128, 150], F32, tag="po", name="po")
                nc.tensor.transpose(po[0:tsz[g], 0:128], HmF[0][:, cs],
                                    Ifp[0:128, 0:128])
                nc.tensor.transpose(po[0:tsz[g], 128:150], HmF[1][0:22, cs],
                                    Ifp[0:22, 0:22])
                nc.vector.tensor_copy(hrR[g][:], po[0:tsz[g], 0:150])
                nc.sync.dma_start(
                    hr_d.ap()[1 + 128 * g:1 + 128 * g + tsz[g], :],
                    hrR[g][0:tsz[g], :])

    nc.compile()
    return nc


def _bf(x):
    return np.ascontiguousarray(np.asarray(x, np.float32)).astype(BF)


def prep_shared(E, Wq, Wp, Wr, w, ctx_Wih, ctx_Whh, ctx_bih, ctx_bhh,
                q_Wih, q_Whh, q_bih, q_bhh, m_Wih, m_Whh, m_bih, m_bhh):
    f = {}
    f["Ifp"] = np.eye(128, dtype=np.float32)
    f["Ibf"] = _bf(np.eye(128))
    f["ones_row"] = _bf(np.ones((1, 512)))

    def wih_chunks(pfx, Wih, bih):
        WT = np.asarray(Wih, np.float32).T  # [300, 450]
        f[f"WihT_{pfx}_0"] = _bf(WT[0:128])
        f[f"WihT_{pfx}_1"] = _bf(WT[128:256])
        f[f"WihT_{pfx}_2"] = _bf(np.vstack([WT[256:300],
                                            np.asarray(bih, np.float32)[None, :]]))

    def whh_chunks(pfx, Whh, bhh):
        WT = np.asarray(Whh, np.float32).T  # [150, 450]
        f[f"WhhT_{pfx}_0"] = _bf(WT[0:128])
        f[f"WhhT_{pfx}_1"] = _bf(np.vstack([WT[128:150],
                                            np.asarray(bhh, np.float32)[None, :]]))

    def sq_chunks(pfx, M, brow=None):
        M = np.asarray(M, np.float32)
        if brow is None:
            brow = np.zeros((1, M.shape[1]), np.float32)
        f[f"{pfx}_0"] = _bf(M[0:128])
        f[f"{pfx}_1"] = _bf(np.vstack([M[128:150], brow]))

    wih_chunks("q", q_Wih, q_bih)
    wih_chunks("c", ctx_Wih, ctx_bih)
    whh_chunks("q", q_Whh, q_bhh)
    whh_chunks("c", ctx_Whh, ctx_bhh)
    whh_chunks("m", m_Whh, m_bhh)
    m_Wih = np.asarray(m_Wih, np.float32)
    # m_bih rides on WcT's bias row (paired with the ones row of HcC chunk 1)
    sq_chunks("WcT", m_Wih[:, :H].T, np.asarray(m_bih, np.float32)[None, :])
    sq_chunks("W2T", m_Wih[:, H:].T)
    sq_chunks("Wr", np.asarray(Wr, np.float32))
    sq_chunks("Wp", np.asarray(Wp, np.float32))
    sq_chunks("Wq", np.asarray(Wq, np.float32))
    wf = np.asarray(w, np.float32)
    f["w_col_0"] = _bf(wf[0:128][:, None])
    f["w_col_1"] = _bf(wf[128:150][:, None])
    return f


_NC_CACHE = {}


def kernel(context, query, E, Wq, Wp, Wr, w, ctx_Wih, ctx_Whh, ctx_bih,
           ctx_bhh, q_Wih, q_Whh, q_bih, q_bhh, m_Wih, m_Whh, m_bih, m_bhh,
           _T=None):
    context = np.asarray(context)
    query = np.asarray(query)
    B, T = context.shape
    if _T is not None:
        T = _T
        context = context[:, :T]
    NT = math.ceil(T / 128)
    if T not in _NC_CACHE:
        _NC_CACHE[T] = build(T)
    nc = _NC_CACHE[T]

    shared = prep_shared(E, Wq, Wp, Wr, w, ctx_Wih, ctx_Whh, ctx_bih, ctx_bhh,
                         q_Wih, q_Whh, q_bih, q_bhh, m_Wih, m_Whh, m_bih, m_bhh)
    E_np = np.ascontiguousarray(np.asarray(E, np.float32))
    in_maps = []
    for b in range(B):
        m = dict(shared)
        m["E"] = E_np
        ci = np.zeros((128, NT), np.int32)
        flat = np.asarray(context[b], np.int64).astype(np.int32)
        for g in range(NT):
            n = min(128, T - 128 * g)
            ci[0:n, g] = flat[128 * g:128 * g + n]
        m["ctx_idx"] = ci
        m["q_idx"] = np.asarray(query[b], np.int64).astype(np.int32)[:, None]
        in_maps.append(m)

    res = run_bass_kernel_spmd(nc, in_maps, core_ids=list(range(B)))
    out = np.stack([r["hr"] for r in res.results], axis=0)
    return out.astype(np.float32)
